# revision 39
# baseline (speedup 1.0000x reference)
"""Trainium2 Bass kernel for nn_MultiHeadAttention_38027640439053.

Reference computation (per batch b of 8, one NeuronCore each):
    data = X.reshape(n, 16, 64)
    q/k/v = data @ W{q,k,v}.T          (per-head shared 64x64 weights)
    scores = (q @ k.T per head) / 32
    attn = softmax(scores, axis=k)
    Y = (attn @ v).reshape(n, 1024) @ Wo.T + bo

Strategy (batch-parallel over 8 cores, zero collectives), v3:
  - All matmul operands bf16 (tolerance 2e-2; we land ~5e-3). X is
    transposed + cast on the host, so no on-chip transposes.
  - Per head-pair p: qt/kt = blockdiag(W.T) @ xt; transposed scores
    ST[k,q] = K Q^T per head (two heads row-tiled concurrently, fp32
    PSUM); exp on ScalarE from PSUM (scale 1/32 folded), bf16 out.
    The score loop is ACT(exp)-bound, so each k-tile issues its score
    matmuls FIRST, then a 4-matmul chunk of the previous pair's P@V
    and a slice of the next pair's projections to fill the PE.
  - P@V: YT = V^T P^T, V row-layout + ones column -> pvps row 64 is
    the softmax denominator D.
  - Normalization via reciprocal_approx_fast (single custom-DVE op,
    ~5x faster than nc.vector.reciprocal's 6.5us/call; plain
    ActivationFunctionType.Reciprocal is hard-blocked in bass).
    Pairs 0-5 batch through rd rows (one [12,N] recip issued at pair
    7's kt0) and normalize inside pair 7's score loop; pair 6 batches
    at the tail via rd2; pair 7 stages D rows to partition 0 (the
    custom op ignores src partition offsets) and its head 1
    accumulates P@V in two 1-bank mps tiles so it never waits on
    head 0's finish chain.
  - Output projection (p-ascending, pair 7 last) overlaps the tail's
    P@V; bias via rank-1 (ones x bo) matmul in each accumulation
    group, skipped when bo is all zeros. Constants arrive in one
    packed DMA blob.
"""

import numpy as np
import ml_dtypes

import concourse.bacc as bacc
import concourse.mybir as mybir
import concourse.tile as tile
from concourse.bass_utils import run_bass_kernel_spmd

F32 = mybir.dt.float32
BF16 = mybir.dt.bfloat16

EXP = mybir.ActivationFunctionType.Exp

# const blob column layout (bf16, [128, 2944])
CB_WQ, CB_WK, CB_WV = 0, 128, 256
CB_SELH = 384        # [1, 256]: selh0 cols 384:512, selh1 cols 512:640
CB_ONES = 640        # [1, 128]
CB_SEL = 768         # [12, 1024] (pairs 0-5 broadcast selector)
CB_BO = 1792         # [1, 1024]
CB_SEL2 = 2816       # [2, 128] (pair 6 broadcast selector)
CB_COLS = 2944


def emit_body(tc, nc, aps, N, EMB, NH, rep, has_bias=True):
    NPAIR = NH // 2
    KT = N // 128
    NT = N // 128
    assert EMB == NPAIR * 128
    scale = 1.0 / float(np.sqrt(EMB))
    qch = [(s, min(512, N - s)) for s in range(0, N, 512)]

    XT_d, CB_d, WoT_d, Y_d = aps

    with (
        tc.tile_pool(name=f"consts{rep}", bufs=1) as consts,
        tc.tile_pool(name=f"xtp{rep}", bufs=2) as xtp,
        tc.tile_pool(name=f"qkp{rep}", bufs=4) as qkp,
        tc.tile_pool(name=f"vp{rep}", bufs=2) as vp,
        tc.tile_pool(name=f"ptp{rep}", bufs=2) as ptp,
        tc.tile_pool(name=f"ytp{rep}", bufs=NPAIR) as ytp,
        tc.tile_pool(name=f"rdp{rep}", bufs=1) as rdp,
        tc.tile_pool(name=f"dhp{rep}", bufs=2) as dhp,
        tc.tile_pool(name=f"osp{rep}", bufs=2) as osp,
        tc.tile_pool(name=f"stps{rep}", bufs=2, space="PSUM") as stps,
        tc.tile_pool(name=f"pvps{rep}", bufs=1, space="PSUM") as pvp,
        tc.tile_pool(name=f"mps{rep}", bufs=2, space="PSUM") as mps,
    ):
        # ~10 dummy matmuls on a never-written scratch tile: keeps the
        # PE busy through the input-DMA window so HAM reaches full clock
        # before the first real matmul (reads garbage, output never read)
        scr = consts.tile([128, 512], BF16, name="scr", tag="scr")
        nc.vector.memset(scr[:], 0.0)
        for _ in range(10):
            wps = mps.tile([128, 512], F32, name="warm", tag="m")
            nc.tensor.matmul(wps[:], scr[:, 0:128], scr[:])
        cb = consts.tile([128, CB_COLS], BF16, name="cb", tag="cb")
        nc.sync.dma_start(out=cb[:], in_=CB_d[:])
        wq2 = cb[:, CB_WQ:CB_WQ + 128]
        wk2 = cb[:, CB_WK:CB_WK + 128]
        wv2 = cb[:, CB_WV:CB_WV + 128]
        selh = [cb[0:1, CB_SELH + 128 * h:CB_SELH + 128 * (h + 1)]
                for h in (0, 1)]
        ones_t = cb[0:1, CB_ONES:CB_ONES + 128]
        sel_t = cb[0:12, CB_SEL:CB_SEL + EMB]
        bo_t = cb[0:1, CB_BO:CB_BO + EMB]
        sel2 = cb[0:2, CB_SEL2:CB_SEL2 + 128]

        rd = rdp.tile([12, N], F32, name="rd", tag="rd")
        rdf = rdp.tile([12, N], F32, name="rdf", tag="rdf")
        rdinv = rdp.tile([12, N], BF16, name="rdinv", tag="rdinv")
        rd2 = rdp.tile([2, N], F32, name="rd2", tag="rd2")
        rdf2 = rdp.tile([2, N], F32, name="rdf2", tag="rdf2")
        rdinv2 = rdp.tile([2, N], BF16, name="rdinv2", tag="rdinv2")

        xts = {}

        def load_xt(p):
            xt = xtp.tile([128, N], BF16, name=f"xt{p}", tag="xt")
            nc.sync.dma_start(out=xt[:], in_=XT_d[p * 128:(p + 1) * 128, :])
            xts[p] = xt

        def proj_one(p, w, nm):
            xt = xts[p]
            dst = qkp.tile([128, N], BF16, name=f"{nm}{p}", tag="qk")
            for (s, ww) in qch:
                ps = mps.tile([128, 512], F32, name=f"p{nm}{p}_{s}", tag="m")
                nc.tensor.matmul(ps[:], w, xt[:, s:s + ww])
                nc.vector.tensor_copy(dst[:, s:s + ww], ps[:])
            return dst

        def proj_v_half(p, half):
            # V in row layout: [n, 2 heads x (64 dims + ones col)]
            if half == 0:
                vslabs[p] = vp.tile([128, KT * 130], BF16,
                                    name=f"vslab{p}", tag="v")
            vslab = vslabs[p]
            xt = xts[p]
            ps = mps.tile([128, 512], F32, name=f"vps{p}_{half}", tag="m")
            for i in range(4):
                j = half * 4 + i
                nc.tensor.matmul(ps[:, i * 128:(i + 1) * 128],
                                 xt[:, j * 128:(j + 1) * 128], wv2)
            vv = vslab[:, half * 4 * 130:(half * 4 + 4) * 130] \
                .rearrange("p (j c) -> p j c", c=130)
            vs = ps[:].rearrange("p (j c) -> p j c", c=128)
            nc.vector.tensor_copy(vv[:, :, 0:64], vs[:, :, 0:64])
            nc.vector.tensor_copy(vv[:, :, 65:129], vs[:, :, 64:128])
            v4 = vv.rearrange("p j (k c) -> p j k c", c=65)
            nc.vector.memset(v4[:, :, :, 64:65], 1.0)
            if half == 1:
                xts.pop(p)

        def st_exp(p, ktile, qt, kt, pt):
            """Transposed scores + exp for one k-tile, both heads."""
            for head in (0, 1):
                r0 = head * 64
                st = stps.tile([128, N], F32, name=f"st{p}_{ktile}_{head}",
                               tag="st")
                for (s, w) in qch:
                    nc.tensor.matmul(
                        st[:, s:s + w],
                        kt[r0:r0 + 64, ktile * 128:(ktile + 1) * 128],
                        qt[r0:r0 + 64, s:s + w],
                    )
                dst = pt[:, (ktile * 2 + head) * N:(ktile * 2 + head + 1) * N]
                nc.scalar.activation(dst, st[:], EXP, scale=scale)

        pv_state = {}

        def pv_chunk(p, j, vslab, pt):
            """4 accumulating matmuls; chunks 0-3 = head 0, 4-7 = head 1."""
            head = j // 4
            if j % 4 == 0:
                pv_state[(p, head)] = pvp.tile(
                    [65, N], F32, name=f"pvps{p}_{head}", tag="pv")
            pvps = pv_state[(p, head)]
            for t in range(4):
                e = (j % 4) * 4 + t
                kv, (s, w) = e // 2, qch[e % 2]
                lhs = vslab[:, kv * 130 + head * 65:kv * 130 + head * 65 + 65]
                base = (kv * 2 + head) * N
                nc.tensor.matmul(
                    pvps[:, s:s + w], lhs, pt[:, base + s:base + s + w],
                    start=(kv == 0), stop=(kv == KT - 1),
                )

        def finish_head(p, head, yt):
            """CAST the head's YT out of PSUM; return pvps (D row live)."""
            pvps = pv_state.pop((p, head))
            nc.vector.tensor_copy(yt[head * 64:head * 64 + 64, :],
                                  pvps[0:64, :])
            return pvps

        def d_row(p, head, pvps):
            """Pairs 0-6: denominator -> rd/rd2 row via partition-0 staging."""
            dh = dhp.tile([1, N], F32, name=f"dh{p}_{head}", tag="dh")
            nc.vector.tensor_copy(dh[:], pvps[64:65, :])
            if p < NPAIR - 2:
                nc.sync.dma_start(out=rd[2 * p + head:2 * p + head + 1, :],
                                  in_=dh[:])
            else:
                nc.sync.dma_start(out=rd2[head:head + 1, :], in_=dh[:])

        def d_inline(p, head, pvps):
            """Pair 7: Dinv via fast DVE reciprocal. The custom-DVE op
            ignores src partition offsets, so stage D at partition 0."""
            dhf = dhp.tile([1, N], F32, name=f"dhf{p}_{head}", tag=f"dhf{head}")
            dvf = dhp.tile([1, N], F32, name=f"dvf{p}_{head}", tag=f"dvf{head}")
            dv = dhp.tile([1, N], BF16, name=f"dv{p}_{head}", tag=f"dv{head}")
            nc.vector.tensor_copy(dhf[:], pvps[64:65, :])
            nc.vector.reciprocal_approx_fast(out=dvf[:], in_=dhf[:])
            nc.vector.tensor_copy(dv[:], dvf[:])
            d_invs[(p, head)] = dv

        def norm_batch():
            # ~51-ULP reciprocal, ~5x faster than nc.vector.reciprocal
            nc.vector.reciprocal_approx_fast(out=rdf[:], in_=rd[:])
            nc.vector.tensor_copy(rdinv[:], rdf[:])

        def norm_batch2():
            nc.vector.reciprocal_approx_fast(out=rdf2[:], in_=rd2[:])
            nc.vector.tensor_copy(rdinv2[:], rdf2[:])

        def bcast_mul(p, pool=None, tag="m"):
            pool = pool if pool is not None else mps
            yt = yts[p]
            for (s, w) in qch:
                bps = pool.tile([128, 512], F32, name=f"bps{p}_{s}", tag=tag)
                if p < NPAIR - 2:
                    nc.tensor.matmul(bps[:], sel_t[:, p * 128:(p + 1) * 128],
                                     rdinv[:, s:s + w])
                elif p == NPAIR - 2:
                    nc.tensor.matmul(bps[:], sel2, rdinv2[:, s:s + w])
                else:
                    nc.tensor.matmul(bps[:], selh[0], d_invs[(p, 0)][:, s:s + w],
                                     start=True, stop=False)
                    nc.tensor.matmul(bps[:], selh[1], d_invs[(p, 1)][:, s:s + w],
                                     start=False, stop=True)
                nc.vector.tensor_mul(yt[:, s:s + w], yt[:, s:s + w], bps[:])

        # ---------------- pipelined pair loop ----------------
        yts = []
        pts = {}
        vslabs = {}
        d_invs = {}

        load_xt(0)
        cur_qt = proj_one(0, wq2, "qt")
        cur_kt = proj_one(0, wk2, "kt")
        proj_v_half(0, 0)
        proj_v_half(0, 1)
        for p in range(NPAIR):
            pt = ptp.tile([128, 2 * KT * N], BF16, name=f"pt{p}", tag="pt")
            pts[p] = pt
            yts.append(ytp.tile([128, N], BF16, name=f"yt{p}", tag="yt"))
            lastp = p == NPAIR - 1

            sched = {k: [] for k in range(KT)}
            if p > 0:
                po, vo, po_pt, yo = p - 1, vslabs[p - 1], pts[p - 1], yts[p - 1]

                def fin(head):
                    pvps = finish_head(po, head, yo)
                    d_row(po, head, pvps)

                # chunk 7 lands at kt6 (not kt7) so the next pair's PT slot
                # (WAR on this PT's last read) frees before its first exp;
                # kt7 is left light so the PE catches up before the boundary
                for j, pos in enumerate((0, 1, 2, 3, 4, 5, 6, 6)):
                    sched[pos].append(lambda j=j: pv_chunk(po, j, vo, po_pt))
                sched[3].append(lambda: fin(0))
                sched[7].append(lambda: fin(1))
            if p + 1 < NPAIR:
                pn = p + 1
                sched[0].append(lambda: load_xt(pn))
                sched[1].append(
                    lambda: nxt.__setitem__("qt", proj_one(pn, wq2, "qt")))
                sched[2].append(
                    lambda: nxt.__setitem__("kt", proj_one(pn, wk2, "kt")))
                sched[4].append(lambda: proj_v_half(pn, 0))
                sched[5].append(lambda: proj_v_half(pn, 1))
            if lastp:
                # recip first in the DVE queue, then all six batch-pair
                # normalizations behind it (their muls only gate outproj).
                # Keep muls OFF kts 3 and 7: those carry the pair-6 finish
                # casts, and overloading the DVE there backs the bps-matmul
                # WAR chain into the PE FIFO, starving late exps.
                sched[0].append(norm_batch)
                for j, pos in enumerate((1, 2, 4, 5, 6, 6)):
                    sched[pos].append(lambda j=j: bcast_mul(j))
            nxt = {}
            for ktile in range(KT):
                st_exp(p, ktile, cur_qt, cur_kt, pt)
                for t in sched[ktile]:
                    t()
            if p - 1 >= 0:
                del vslabs[p - 1], pts[p - 1]
            if p + 1 < NPAIR:
                cur_qt, cur_kt = nxt["qt"], nxt["kt"]

        # ---------------- tail ----------------
        last = NPAIR - 1
        # Wo^T streams into a recycled PT slot while last pair's PV runs
        wot = ptp.tile([128, NPAIR * EMB], BF16, name="wot", tag="pt")
        nc.sync.dma_start(
            out=wot[:].rearrange("p (c e) -> p c e", e=EMB),
            in_=WoT_d[:].rearrange("(c p) e -> p c e", p=128))
        # PE: both heads' P@V back-to-back. Head 1 accumulates in two
        # 1-bank mps tiles so it does not wait (WAR) on head 0's finish
        # chain freeing the pvp slot. All tail bps/ops PSUM comes from the
        # now-idle score pool (stps, same tag = no extra banks), so
        # nothing here waits on the DVE mul chain through mps slots.
        for j in range(4):
            pv_chunk(last, j, vslabs[last], pts[last])
        vo, po_pt = vslabs[last], pts[last]
        pv1 = [mps.tile([65, 512], F32, name=f"pv1_{s}", tag="m")
               for (s, _) in qch]
        for kv in range(KT):
            lhs = vo[:, kv * 130 + 65:kv * 130 + 130]
            base = (kv * 2 + 1) * N
            for qi, (s, w) in enumerate(qch):
                nc.tensor.matmul(pv1[qi][:], lhs,
                                 po_pt[:, base + s:base + s + w],
                                 start=(kv == 0), stop=(kv == KT - 1))
        # DVE queue order matters (strict FIFO): both heads' D staging +
        # reciprocals go FIRST -- they gate the final broadcasts and the
        # output projection's pair-7 matmuls. YT casts and muls follow.
        pvps0 = pv_state.pop((last, 0))
        dhf0 = dhp.tile([1, N], F32, name="dhf7_0", tag="dhf0")
        nc.vector.tensor_copy(dhf0[:], pvps0[64:65, :])
        dhf1 = dhp.tile([1, N], F32, name="dhf7_1", tag="dhf1")
        for qi, (s, w) in enumerate(qch):
            nc.vector.tensor_copy(dhf1[:, s:s + w], pv1[qi][64:65, :])
        dvf0 = dhp.tile([1, N], F32, name="dvf7_0", tag="dvf0")
        dv0 = dhp.tile([1, N], BF16, name="dv7_0", tag="dv0")
        nc.vector.reciprocal_approx_fast(out=dvf0[:], in_=dhf0[:])
        nc.vector.tensor_copy(dv0[:], dvf0[:])
        d_invs[(last, 0)] = dv0
        dvf1 = dhp.tile([1, N], F32, name="dvf7_1", tag="dvf1")
        dv1 = dhp.tile([1, N], BF16, name="dv7_1", tag="dv1")
        nc.vector.reciprocal_approx_fast(out=dvf1[:], in_=dhf1[:])
        nc.vector.tensor_copy(dv1[:], dvf1[:])
        d_invs[(last, 1)] = dv1
        norm_batch2()
        nc.vector.tensor_copy(yts[last][0:64, :], pvps0[0:64, :])
        for qi, (s, w) in enumerate(qch):
            nc.vector.tensor_copy(yts[last][64:128, s:s + w],
                                  pv1[qi][0:64, :])
        # open the first output n-tile's two chunks early: their pair-0..5
        # matmuls run on the PE while pair 6/7's normalization chains are
        # still in flight on the DVE
        ops0 = {}
        for (s, w) in qch:
            ops = stps.tile([128, 512], F32, name=f"ops0_{s}", tag="st")
            ops0[s] = ops
            for p in range(NPAIR - 2):
                nc.tensor.matmul(ops[:], yts[p][:, 0:128],
                                 wot[:, p * EMB + s:p * EMB + s + w],
                                 start=(p == 0), stop=False)
        bcast_mul(NPAIR - 2)
        bcast_mul(NPAIR - 1)

        # output projection: Y[i-tile] = sum_p yt_p^T @ WoT_p + bo
        for i in range(NT):
            osb = osp.tile([128, EMB], F32, name=f"osb{i}", tag="o")
            for (s, w) in qch:
                if i == 0:
                    ops, p0 = ops0[s], NPAIR - 2
                else:
                    ops = stps.tile([128, 512], F32, name=f"ops{i}_{s}",
                                    tag="st")
                    p0 = 0
                for p in range(p0, NPAIR):
                    nc.tensor.matmul(
                        ops[:],
                        yts[p][:, i * 128:(i + 1) * 128],
                        wot[:, p * EMB + s:p * EMB + s + w],
                        start=(p == 0), stop=(not has_bias and p == NPAIR - 1),
                    )
                if has_bias:
                    nc.tensor.matmul(ops[:], ones_t, bo_t[:, s:s + w],
                                     start=False, stop=True)
                nc.vector.tensor_copy(osb[:, s:s + w], ops[:])
            nc.sync.dma_start(out=Y_d[i * 128:(i + 1) * 128, :], in_=osb[:])


def build_program(N=1024, EMB=1024, NH=16, n_cores=8, repeat=1,
                  trace_sim=False, has_bias=True):
    nc = bacc.Bacc("TRN2", target_bir_lowering=False, debug=False,
                   num_devices=n_cores)
    aps = (
        nc.dram_tensor("XT", [EMB, N], BF16, kind="ExternalInput").ap(),
        nc.dram_tensor("CB", [128, CB_COLS], BF16, kind="ExternalInput").ap(),
        nc.dram_tensor("WoT", [EMB, EMB], BF16, kind="ExternalInput").ap(),
        nc.dram_tensor("Y", [N, EMB], F32, kind="ExternalOutput").ap(),
    )
    with tile.TileContext(nc, trace_sim=trace_sim) as tc:
        for rep in range(repeat):
            emit_body(tc, nc, aps, N, EMB, NH, rep, has_bias=has_bias)
    nc.compile()
    return nc


def host_consts(Wq, Wk, Wv, Wo, bo, NH=16):
    EMB = NH * 64
    bf = ml_dtypes.bfloat16

    cbf = np.zeros((128, CB_COLS), np.float32)

    def blk2(W):
        out = np.zeros((128, 128), np.float32)
        out[0:64, 0:64] = W.T
        out[64:128, 64:128] = W.T
        return out

    cbf[:, CB_WQ:CB_WQ + 128] = blk2(np.asarray(Wq, np.float32))
    cbf[:, CB_WK:CB_WK + 128] = blk2(np.asarray(Wk, np.float32))
    cbf[:, CB_WV:CB_WV + 128] = blk2(np.asarray(Wv, np.float32))
    cbf[0, CB_SELH:CB_SELH + 64] = 1.0          # selh0
    cbf[0, CB_SELH + 192:CB_SELH + 256] = 1.0   # selh1
    cbf[0, CB_ONES:CB_ONES + 128] = 1.0
    for p in range(NH // 2 - 2):                # pairs 0-5 selector
        cbf[2 * p, CB_SEL + p * 128:CB_SEL + p * 128 + 64] = 1.0
        cbf[2 * p + 1, CB_SEL + p * 128 + 64:CB_SEL + p * 128 + 128] = 1.0
    cbf[0, CB_BO:CB_BO + EMB] = np.asarray(bo, np.float32)
    cbf[0, CB_SEL2:CB_SEL2 + 64] = 1.0          # pair-6 selector
    cbf[1, CB_SEL2 + 64:CB_SEL2 + 128] = 1.0
    return {
        "CB": cbf.astype(bf),
        "WoT": np.ascontiguousarray(np.asarray(Wo, np.float32).T).astype(bf),
    }


_NC_CACHE = {}


def kernel(X, Wq, Wk, Wv, Wo, bo):
    X = np.asarray(X, np.float32)
    B, N, EMB = X.shape
    NH = EMB // 64
    has_bias = bool(np.any(np.asarray(bo)))
    key = (N, EMB, NH, B, has_bias)
    if key not in _NC_CACHE:
        _NC_CACHE[key] = build_program(N=N, EMB=EMB, NH=NH, n_cores=B,
                                       has_bias=has_bias)
    nc = _NC_CACHE[key]
    consts = host_consts(Wq, Wk, Wv, Wo, bo, NH=NH)
    bf = ml_dtypes.bfloat16
    in_maps = [
        dict(consts, XT=np.ascontiguousarray(X[c].T).astype(bf))
        for c in range(B)
    ]
    res = run_bass_kernel_spmd(nc, in_maps, list(range(B)))
    return np.stack([res.results[c]["Y"] for c in range(B)], axis=0)


if __name__ == "__main__":
    rng = np.random.default_rng(0)
    B, N, EMB, NH = 8, 1024, 1024, 16
    X = rng.standard_normal((B, N, EMB), dtype=np.float32)
    Wq = (rng.standard_normal((64, 64), dtype=np.float32) / 8)
    Wk = (rng.standard_normal((64, 64), dtype=np.float32) / 8)
    Wv = (rng.standard_normal((64, 64), dtype=np.float32) / 8)
    Wo = (rng.standard_normal((EMB, EMB), dtype=np.float32) / 32)
    bo = np.zeros(EMB, np.float32)
    Y = kernel(X=X, Wq=Wq, Wk=Wk, Wv=Wv, Wo=Wo, bo=bo)

    def ref(X, Wq, Wk, Wv, Wo, bo):
        b, n, d = X.shape
        hd = Wq.shape[0]
        h = d // hd
        data = X.reshape(b, n, h, hd)
        q = np.einsum('bnhd,ed->bnhe', data, Wq)
        k = np.einsum('bnhd,ed->bnhe', data, Wk)
        v = np.einsum('bnhd,ed->bnhe', data, Wv)
        s = np.einsum('bqhd,bkhd->bhqk', q, k) / np.sqrt(d)
        s = np.exp(s - s.max(-1, keepdims=True))
        attn = s / s.sum(-1, keepdims=True)
        Yr = np.einsum('bhqk,bkhd->bqhd', attn, v).reshape(b, n, d)
        return Yr @ Wo.T + bo

    E = ref(X, Wq, Wk, Wv, Wo, bo)
    err = np.abs(Y - E).max() / np.abs(E).max()
    print("OK", Y.shape, Y.dtype, "rel err", err)
